# revision 1
# baseline (speedup 1.0000x reference)
"""AdaptiveStructureBlock kernel: data-parallel over batch across 8 NeuronCores.

Strategy (per sharding hint): batch B=8 is split 1-per-core. Every op
(cosine sim, topk, scatter, bmm, conv) is batch-independent except the
BatchNorm statistics, which are computed over the full batch via a
cross-device mean (psum over the device axis).
"""

import numpy as np

SPARSITY = 0.02
K_NEIGHBORS = 16
BN_EPS = 1e-5

B, N, D = 8, 1024, 768
N_CORES = 8

_COMPILED = {}


def _build(backend):
    import jax
    import jax.numpy as jnp
    from functools import partial

    devs = jax.devices(backend)[:N_CORES]

    def per_core(x, gcn_weight, gcn_bias, conv1_w, conv1_b, bn_gamma, bn_beta,
                 conv2_w, conv2_b):
        # x: [1, N, D] local batch shard
        xn = x / jnp.maximum(jnp.linalg.norm(x, axis=-1, keepdims=True), 1e-12)
        sim = jnp.einsum('bnd,bmd->bnm', xn, xn)
        k = min(K_NEIGHBORS, N)
        vals, idx = jax.lax.top_k(sim, k)
        bi = jnp.arange(x.shape[0])[:, None, None]
        ri = jnp.arange(N)[None, :, None]
        mask = jnp.zeros_like(sim).at[bi, ri, idx].set(vals)
        adj = (mask + jnp.swapaxes(mask, 1, 2)) * 0.5

        qw = (jnp.where(gcn_weight > SPARSITY, 1.0, 0.0)
              - jnp.where(gcn_weight < -SPARSITY, 1.0, 0.0)) * SPARSITY
        support = jnp.einsum('bnd,de->bne', x, qw)
        gcn_out = jax.nn.relu(jnp.einsum('bnm,bme->bne', adj, support) + gcn_bias)

        h = jax.lax.conv_general_dilated(
            x[:, None, :, :], conv1_w, (1, 1), ((1, 1), (1, 1)),
            dimension_numbers=('NCHW', 'OIHW', 'NCHW'))
        h = h + conv1_b[None, :, None, None]
        # cross-device batch statistics
        mu = jax.lax.pmean(jnp.mean(h, axis=(0, 2, 3)), axis_name='i')
        e2 = jax.lax.pmean(jnp.mean(h * h, axis=(0, 2, 3)), axis_name='i')
        var = e2 - mu * mu
        h = (h - mu[None, :, None, None]) / jnp.sqrt(var[None, :, None, None] + BN_EPS)
        h = h * bn_gamma[None, :, None, None] + bn_beta[None, :, None, None]
        h = jax.nn.relu(h)
        conv_out = jax.lax.conv_general_dilated(
            h, conv2_w, (1, 1), ((1, 1), (1, 1)),
            dimension_numbers=('NCHW', 'OIHW', 'NCHW'))
        conv_out = (conv_out + conv2_b[None, :, None, None])[:, 0]
        return gcn_out + conv_out

    fn = jax.pmap(per_core, axis_name='i', devices=devs,
                  in_axes=(0, None, None, None, None, None, None, None, None))
    return fn


def kernel(x, gcn_weight, gcn_bias, conv1_w, conv1_b, bn_gamma, bn_beta,
           conv2_w, conv2_b):
    import jax

    x = np.asarray(x, dtype=np.float32)
    args = [np.asarray(a, dtype=np.float32) for a in
            (gcn_weight, gcn_bias, conv1_w, conv1_b, bn_gamma, bn_beta,
             conv2_w, conv2_b)]

    # batch shards: [n_cores, B/n_cores, N, D]
    xs = x.reshape(N_CORES, B // N_CORES, N, D)

    for backend in ("axon", "neuron", None):
        try:
            key = backend
            if key not in _COMPILED:
                _COMPILED[key] = _build(backend)
            out = _COMPILED[key](xs, *args)
            out = np.asarray(out, dtype=np.float32).reshape(B, N, D)
            return out
        except Exception:
            continue

    # pure-numpy fallback (should not be reached)
    return _numpy_ref(x, *args)


def _numpy_ref(x, gcn_weight, gcn_bias, conv1_w, conv1_b, bn_gamma, bn_beta,
               conv2_w, conv2_b):
    xn = x / np.maximum(np.linalg.norm(x, axis=-1, keepdims=True), 1e-12)
    sim = np.einsum('bnd,bmd->bnm', xn, xn)
    k = K_NEIGHBORS
    idx = np.argpartition(-sim, k - 1, axis=-1)[..., :k]
    vals = np.take_along_axis(sim, idx, axis=-1)
    mask = np.zeros_like(sim)
    bi = np.arange(B)[:, None, None]
    ri = np.arange(N)[None, :, None]
    mask[bi, ri, idx] = vals
    adj = (mask + np.swapaxes(mask, 1, 2)) * 0.5
    qw = ((gcn_weight > SPARSITY).astype(np.float32)
          - (gcn_weight < -SPARSITY).astype(np.float32)) * SPARSITY
    support = np.einsum('bnd,de->bne', x, qw)
    gcn_out = np.maximum(np.einsum('bnm,bme->bne', adj, support) + gcn_bias, 0.0)

    def conv2d(img, w, b):
        Bc, C, H, W = img.shape
        O, I, kh, kw = w.shape
        p = np.pad(img, ((0, 0), (0, 0), (1, 1), (1, 1)))
        out = np.zeros((Bc, O, H, W), np.float32)
        for dy in range(3):
            for dx in range(3):
                out += np.einsum('bchw,oc->bohw',
                                 p[:, :, dy:dy + H, dx:dx + W], w[:, :, dy, dx])
        return out + b[None, :, None, None]

    h = conv2d(x[:, None], conv1_w, conv1_b)
    mu = h.mean(axis=(0, 2, 3), keepdims=True)
    var = ((h - mu) ** 2).mean(axis=(0, 2, 3), keepdims=True)
    h = (h - mu) / np.sqrt(var + BN_EPS)
    h = h * bn_gamma[None, :, None, None] + bn_beta[None, :, None, None]
    h = np.maximum(h, 0.0)
    conv_out = conv2d(h, conv2_w, conv2_b)[:, 0]
    return (gcn_out + conv_out).astype(np.float32)
